# revision 10
# baseline (speedup 1.0000x reference)
"""Bass/Trainium2 kernel for the additive-attention nn.Module.

Computation (per batch b):
    energy[s, a] = tanh( enc[b,s,:] @ W_enc[a,:] + hidden[b,:] @ W_dec[a,:] + b_attn[a] )
    scores[s]    = energy[s, :] @ v
    w            = softmax(scores)
    ctx[b, :]    = w @ enc[b]

Sharding: data-parallel over batch across 8 NeuronCores (8 batches/core);
W_attn / b_attn / v replicated.

Per-core kernel layout:
  - W_attn is transposed on-chip (PE transposes) into W_encT [e, a] tiles kept
    in SBUF; the W_dec half is consumed on the fly by the tiny hidden GEMM,
    which together with b_attn produces a per-(a, b) bias table hb.
  - The big GEMM runs as out[a(128), t(512)] += W_encT[e,a].T @ encT[e,t] in
    float32r (fp32 data at 1 PE cycle/row; all operand tiles are float32r-
    typed so producers round on write, as the BIR verifier requires).
  - ScalarEngine computes tanh(energy + hb[:, b]) in one op (per-partition
    bias), then PE contracts with v (partition-dim reduction) into scores.
  - Per-batch softmax on one SBUF row; the exp-weights are re-laid into PE
    columns via a tiny DRAM bounce, then ctx = (ew @ enc) * (1/sum) using the
    natural-layout enc tiles kept resident in SBUF (no HBM re-read).
"""

import sys

if "/opt/trn_rl_repo" not in sys.path:
    sys.path.insert(0, "/opt/trn_rl_repo")

import numpy as np

B, S, DEC, ENC, ATTN = 64, 1024, 1024, 1024, 1024
N_CORES = 8
B_LOC = B // N_CORES

_CACHE = {}


def build_nc(B_loc=B_LOC, S_=S, E=ENC, A=ATTN, D=DEC):
    from contextlib import ExitStack

    import concourse.bacc as bacc
    import concourse.tile as tile
    from concourse import mybir
    from concourse.bass import ts
    from concourse.masks import make_identity

    P = 128
    F32 = mybir.dt.float32
    F32R = mybir.dt.float32r
    AF = mybir.ActivationFunctionType
    AX = mybir.AxisListType

    n_tb = S_ // P            # 128-token blocks per batch
    TCW = min(512, S_)        # token-chunk width (matmul moving N)
    n_tc = S_ // TCW
    tb_per_tc = TCW // P
    n_eb = E // P
    n_ab = A // P
    n_db = D // P
    ECW = min(512, E)         # e-chunk width for the context matmul
    n_ec = E // ECW
    AB_GRP = min(2, n_ab)     # a-blocks packed per transpose-psum tile
    F = D + E

    nc = bacc.Bacc("TRN2", target_bir_lowering=False, debug=False)
    hid_d = nc.dram_tensor("hidden", [B_loc, D], F32, kind="ExternalInput")
    enc_d = nc.dram_tensor("enc", [B_loc, S_, E], F32, kind="ExternalInput")
    W_d = nc.dram_tensor("W", [A, F], F32, kind="ExternalInput")
    b_d = nc.dram_tensor("b_attn", [A], F32, kind="ExternalInput")
    v_d = nc.dram_tensor("v", [A], F32, kind="ExternalInput")
    ctx_d = nc.dram_tensor("ctx", [B_loc, E], F32, kind="ExternalOutput")
    ew_d = nc.dram_tensor("ew_scratch", [B_loc, S_], F32)  # internal bounce

    with tile.TileContext(nc) as tc:
        with ExitStack() as ctx:
            const = ctx.enter_context(tc.tile_pool(name="const", bufs=1))
            wnat_p = ctx.enter_context(tc.tile_pool(name="wnat", bufs=AB_GRP))
            wenc_p = ctx.enter_context(tc.tile_pool(name="wenc", bufs=1))
            wdec_p = ctx.enter_context(tc.tile_pool(name="wdec", bufs=2))
            encnat_p = ctx.enter_context(tc.tile_pool(name="encnat", bufs=min(10, B_loc * n_tb)))
            encT_p = ctx.enter_context(tc.tile_pool(name="encT", bufs=2 * n_eb))
            tanh_p = ctx.enter_context(tc.tile_pool(name="tanh", bufs=4))
            soft_p = ctx.enter_context(tc.tile_pool(name="soft", bufs=2))
            psE = ctx.enter_context(tc.tile_pool(name="psE", bufs=2, space="PSUM"))
            psT = ctx.enter_context(tc.tile_pool(name="psT", bufs=2, space="PSUM"))
            psS = ctx.enter_context(tc.tile_pool(name="psS", bufs=1, space="PSUM"))
            psC = ctx.enter_context(tc.tile_pool(name="psC", bufs=1, space="PSUM"))
            psH = ctx.enter_context(tc.tile_pool(name="psH", bufs=2, space="PSUM"))

            # ---- constants ----
            ident0 = const.tile([P, P], F32, name="ident0")
            make_identity(nc, ident0[:])
            ident = const.tile([P, P], F32R, name="ident")
            nc.vector.tensor_copy(ident[:], ident0[:])
            ones_row = const.tile([1, B_loc], F32, name="ones_row")
            nc.gpsimd.memset(ones_row[:], 1.0)
            b_row = const.tile([1, A], F32, name="b_row")
            nc.sync.dma_start(b_row[:], b_d.ap().rearrange("(o a) -> o a", o=1))
            vcol = const.tile([P, n_ab], F32R, name="vcol")
            nc.sync.dma_start(vcol[:], v_d.ap().rearrange("(j p) -> p j", p=P).bitcast(F32R))
            # hidden as [d, db, b] columns, straight from DRAM
            hidT = const.tile([P, n_db, B_loc], F32R, name="hidT")
            for db in range(n_db):
                nc.sync.dma_start(
                    hidT[:, db],
                    hid_d.ap()[:, ts(db, P)].rearrange("b p -> p b").bitcast(F32R),
                )

            # ---- W transpose + hb[a, b] = W_dec @ hidden.T + b_attn ----
            wenc_tiles = []
            for eb in range(n_eb):
                wt = wenc_p.tile([P, A], F32R, tag=f"wenc{eb}", name=f"wenc{eb}")
                wenc_tiles.append(wt)
            hb_all = const.tile([P, n_ab * B_loc], F32, name="hb_all")

            for abg in range(n_ab // AB_GRP):
                wn_tiles = []
                for abl in range(AB_GRP):
                    ab = abg * AB_GRP + abl
                    wn = wnat_p.tile([P, F], F32R, tag="wnat", name=f"wn{ab}")
                    nc.sync.dma_start(wn[:], W_d.ap()[ts(ab, P), :].bitcast(F32R))
                    wn_tiles.append(wn)
                ps_hb = [
                    psH.tile([P, B_loc], F32, tag="hb", name=f"pshb{abg}_{abl}")
                    for abl in range(AB_GRP)
                ]
                for fb in range(n_db + n_eb):
                    pw = psT.tile([P, AB_GRP * P], F32R, tag="t", name=f"pw{abg}_{fb}")
                    for abl in range(AB_GRP):
                        nc.tensor.transpose(
                            pw[:, ts(abl, P)], wn_tiles[abl][:, ts(fb, P)], ident[:]
                        )
                    if fb < n_db:
                        db = fb
                        wd = wdec_p.tile([P, AB_GRP * P], F32R, tag="wdec", name=f"wd{abg}_{db}")
                        nc.vector.tensor_copy(wd[:], pw[:])
                        for abl in range(AB_GRP):
                            nc.tensor.matmul(
                                ps_hb[abl][:],
                                wd[:, ts(abl, P)],
                                hidT[:, db],
                                start=(db == 0),
                                stop=False,
                                skip_group_check=True,
                            )
                    else:
                        eb = fb - n_db
                        nc.vector.tensor_copy(
                            wenc_tiles[eb][:, abg * AB_GRP * P:(abg + 1) * AB_GRP * P], pw[:]
                        )
                # + b_attn (rank-1, plain fp32)
                for abl in range(AB_GRP):
                    ab = abg * AB_GRP + abl
                    nc.tensor.matmul(
                        ps_hb[abl][:],
                        b_row[0:1, ts(ab, P)],
                        ones_row[:],
                        start=False,
                        stop=True,
                        skip_group_check=True,
                    )
                    nc.vector.tensor_copy(
                        hb_all[:, ab * B_loc:(ab + 1) * B_loc], ps_hb[abl][:]
                    )

            # ---- main loop ----
            for b in range(B_loc):
                scores_row = soft_p.tile([1, S_], F32, tag="scores", name=f"scores{b}")
                encnat = {}
                for tcn in range(n_tc):
                    for tbl in range(tb_per_tc):
                        tb = tcn * tb_per_tc + tbl
                        t_enc = encnat_p.tile([P, E], F32R, tag="encnat", name=f"enc_{b}_{tb}")
                        nc.sync.dma_start(t_enc[:], enc_d.ap()[b, ts(tb, P), :].bitcast(F32R))
                        encnat[tb] = t_enc
                    # transpose enc -> encT [e, t]
                    encT = []
                    for eb in range(n_eb):
                        pt = psT.tile([P, TCW], F32R, tag="t", name=f"pt{b}_{tcn}_{eb}")
                        for tbl in range(tb_per_tc):
                            tb = tcn * tb_per_tc + tbl
                            nc.tensor.transpose(
                                pt[:, ts(tbl, P)], encnat[tb][:, ts(eb, P)], ident[:]
                            )
                        et = encT_p.tile([P, TCW], F32R, tag="encT", name=f"encT{b}_{tcn}_{eb}")
                        nc.vector.tensor_copy(et[:], pt[:])
                        encT.append(et)
                    # energy -> tanh -> scores
                    ps_s = psS.tile([1, TCW], F32, tag="s", name=f"pss{b}_{tcn}")
                    for ab in range(n_ab):
                        ps_e = psE.tile([P, TCW], F32, tag="e", name=f"pse{b}_{tcn}_{ab}")
                        for eb in range(n_eb):
                            nc.tensor.matmul(
                                ps_e[:],
                                wenc_tiles[eb][:, ts(ab, P)],
                                encT[eb][:],
                                start=(eb == 0),
                                stop=(eb == n_eb - 1),
                            )
                        th = tanh_p.tile([P, TCW], F32R, tag="tanh", name=f"th{b}_{tcn}_{ab}")
                        nc.scalar.activation(
                            th[:], ps_e[:], AF.Tanh,
                            bias=hb_all[:, ab * B_loc + b: ab * B_loc + b + 1],
                        )
                        nc.tensor.matmul(
                            ps_s[:],
                            vcol[:, ab:ab + 1],
                            th[:],
                            start=(ab == 0),
                            stop=(ab == n_ab - 1),
                            skip_group_check=True,
                        )
                    nc.vector.tensor_copy(scores_row[0:1, ts(tcn, TCW)], ps_s[:])

                # ---- per-batch softmax + context ----
                row = scores_row[0:1, :]
                nm = soft_p.tile([1, 1], F32, tag="nm", name=f"nm{b}")
                nc.vector.reduce_max(nm[:], row, axis=AX.X, negate=True)
                ew = soft_p.tile([1, S_], F32, tag="ew", name=f"ew{b}")
                nc.scalar.activation(ew[:], row, AF.Exp, bias=nm[0:1, 0:1])
                sm = soft_p.tile([1, 1], F32, tag="sm", name=f"sm{b}")
                nc.vector.reduce_sum(sm[:], ew[:], axis=AX.X)
                rc = soft_p.tile([1, 1], F32, tag="rc", name=f"rc{b}")
                nc.vector.reciprocal(rc[:], sm[:])
                # re-lay ew into PE columns via DRAM bounce
                nc.sync.dma_start(ew_d.ap()[b:b + 1, :], ew[:])
                wc = soft_p.tile([P, n_tb], F32R, tag="wc", name=f"wc{b}")
                nc.sync.dma_start(
                    wc[:],
                    ew_d.ap()[b].rearrange("(j p) -> p j", p=P).bitcast(F32R),
                )

                ctx_row = soft_p.tile([1, E], F32, tag="ctxrow", name=f"ctxrow{b}")
                for ec in range(n_ec):
                    ps_c = psC.tile([1, ECW], F32, tag="c", name=f"psc{b}_{ec}")
                    for tb in range(n_tb):
                        nc.tensor.matmul(
                            ps_c[:],
                            wc[:, tb:tb + 1],
                            encnat[tb][:, ts(ec, ECW)],
                            start=(tb == 0),
                            stop=(tb == n_tb - 1),
                            skip_group_check=True,
                        )
                    # scale by 1/sum on the way out of PSUM
                    nc.vector.tensor_scalar_mul(
                        ctx_row[0:1, ts(ec, ECW)], ps_c[:], rc[0:1, 0:1]
                    )
                nc.sync.dma_start(ctx_d.ap()[b:b + 1, :], ctx_row[:])

    nc.compile()
    return nc


def _get_nc():
    key = (B_LOC, S, ENC, ATTN, DEC)
    if key not in _CACHE:
        _CACHE[key] = build_nc(*key)
    return _CACHE[key]


def kernel(hidden, encoder_outputs, W_attn, b_attn, v):
    from concourse.bass_utils import run_bass_kernel_spmd

    hidden = np.ascontiguousarray(np.asarray(hidden, dtype=np.float32))
    enc = np.ascontiguousarray(np.asarray(encoder_outputs, dtype=np.float32))
    W = np.ascontiguousarray(np.asarray(W_attn, dtype=np.float32))
    b = np.ascontiguousarray(np.asarray(b_attn, dtype=np.float32))
    vv = np.ascontiguousarray(np.asarray(v, dtype=np.float32))

    nc = _get_nc()
    in_maps = [
        {
            "hidden": hidden[c * B_LOC:(c + 1) * B_LOC],
            "enc": enc[c * B_LOC:(c + 1) * B_LOC],
            "W": W,
            "b_attn": b,
            "v": vv,
        }
        for c in range(N_CORES)
    ]
    res = run_bass_kernel_spmd(nc, in_maps, core_ids=list(range(N_CORES)))
    out = np.concatenate([res.results[c]["ctx"] for c in range(N_CORES)], axis=0)
    return out.reshape(B, 1, ENC).astype(np.float32)


# revision 21
# speedup vs baseline: 71.8754x; 71.8754x over previous
"""Bass/Trainium2 kernel for the additive-attention nn.Module.

Computation (per batch b):
    energy[s, a] = tanh( enc[b,s,:] @ W_enc[a,:] + hidden[b,:] @ W_dec[a,:] + b_attn[a] )
    scores[s]    = energy[s, :] @ v
    w            = softmax(scores)
    ctx[b, :]    = w @ enc[b]

Sharding: data-parallel over batch across 8 NeuronCores (8 batches/core);
W_attn / b_attn / v replicated.

Per-core kernel layout:
  - W_attn is transposed on-chip (PE transposes) into W_encT [e, a] tiles kept
    in SBUF; the W_dec half is consumed on the fly by the tiny hidden GEMM,
    which together with b_attn produces a per-(a, b) bias table hb.
  - The big GEMM runs as out[a(128), t(512)] += W_encT[e,a].T @ encT[e,t] in
    float32r (fp32 data at 1 PE cycle/row; all operand tiles are float32r-
    typed so producers round on write, as the BIR verifier requires).
  - ScalarEngine computes tanh(energy + hb[:, b]) in one op (per-partition
    bias), then PE contracts with v (partition-dim reduction) into scores.
  - Per-batch softmax on one SBUF row; the exp-weights are re-laid into PE
    columns via a tiny DRAM bounce, then ctx = (ew @ enc) * (1/sum) using the
    natural-layout enc tiles kept resident in SBUF (no HBM re-read).
"""

import sys

if "/opt/trn_rl_repo" not in sys.path:
    sys.path.insert(0, "/opt/trn_rl_repo")

import numpy as np

B, S, DEC, ENC, ATTN = 64, 1024, 1024, 1024, 1024
N_CORES = 8
B_LOC = B // N_CORES

_CACHE = {}


def build_nc(B_loc=B_LOC, S_=S, E=ENC, A=ATTN, D=DEC, loop_n=None):
    from contextlib import ExitStack

    import concourse.bacc as bacc
    import concourse.tile as tile
    from concourse import mybir
    from concourse.bass import ts
    from concourse.masks import make_identity

    P = 128
    F32 = mybir.dt.float32
    F32R = mybir.dt.float32r
    AF = mybir.ActivationFunctionType
    AX = mybir.AxisListType

    n_tb = S_ // P            # 128-token blocks per batch
    TCW = min(512, S_)        # token-chunk width (matmul moving N)
    n_tc = S_ // TCW
    tb_per_tc = TCW // P
    n_eb = E // P
    n_ab = A // P
    n_db = D // P
    ECW = min(512, E)         # e-chunk width for the context matmul
    n_ec = E // ECW
    AB_GRP = min(2, n_ab)     # a-blocks packed per transpose-psum tile
    n_abg = n_ab // AB_GRP
    F = D + E

    nc = bacc.Bacc("TRN2", target_bir_lowering=False, debug=False)
    hid_d = nc.dram_tensor("hidden", [B_loc, D], F32, kind="ExternalInput")
    enc_d = nc.dram_tensor("enc", [B_loc, S_, E], F32, kind="ExternalInput")
    W_d = nc.dram_tensor("W", [A, F], F32, kind="ExternalInput")
    b_d = nc.dram_tensor("b_attn", [A], F32, kind="ExternalInput")
    v_d = nc.dram_tensor("v", [A], F32, kind="ExternalInput")
    ctx_d = nc.dram_tensor("ctx", [B_loc, E], F32, kind="ExternalOutput")
    ew_d = nc.dram_tensor("ew_scratch", [B_loc, S_], F32)  # internal bounce

    with tile.TileContext(nc) as tc:
        with ExitStack() as ctx:
            const = ctx.enter_context(tc.tile_pool(name="const", bufs=1))
            wnat_p = ctx.enter_context(tc.tile_pool(name="wnat", bufs=3 * AB_GRP))
            wenc_p = ctx.enter_context(tc.tile_pool(name="wenc", bufs=1))
            wdec_p = ctx.enter_context(tc.tile_pool(name="wdec", bufs=2))
            encnat_p = ctx.enter_context(tc.tile_pool(name="encnat", bufs=min(16, B_loc * n_tb)))
            encT_p = ctx.enter_context(tc.tile_pool(name="encT", bufs=2 * n_eb))
            tanh_p = ctx.enter_context(tc.tile_pool(name="tanh", bufs=4))
            soft_p = ctx.enter_context(tc.tile_pool(name="soft", bufs=2))
            psE = ctx.enter_context(tc.tile_pool(name="psE", bufs=2, space="PSUM"))
            psT = ctx.enter_context(tc.tile_pool(name="psT", bufs=2, space="PSUM"))
            psS = ctx.enter_context(tc.tile_pool(name="psS", bufs=1, space="PSUM"))
            psC = ctx.enter_context(tc.tile_pool(name="psC", bufs=1, space="PSUM"))
            psH = ctx.enter_context(tc.tile_pool(name="psH", bufs=2, space="PSUM"))

            if loop_n is not None:
                ctx.enter_context(tc.For_i(0, loop_n, 1))

            # ---- constants ----
            ident0 = const.tile([P, P], F32, name="ident0")
            make_identity(nc, ident0[:])
            ident = const.tile([P, P], F32R, name="ident")
            nc.vector.tensor_copy(ident[:], ident0[:])
            ones_row = const.tile([1, B_loc], F32, name="ones_row")
            nc.gpsimd.memset(ones_row[:], 1.0)
            b_row = const.tile([1, A], F32, name="b_row")
            nc.sync.dma_start(b_row[:], b_d.ap().rearrange("(o a) -> o a", o=1))
            vcol = const.tile([P, n_ab], F32R, name="vcol")
            nc.sync.dma_start(vcol[:], v_d.ap().rearrange("(j p) -> p j", p=P).bitcast(F32R))
            # hidden as [d, db, b] columns, straight from DRAM
            hidT = const.tile([P, n_db, B_loc], F32R, name="hidT")
            for db in range(n_db):
                nc.sync.dma_start(
                    hidT[:, db],
                    hid_d.ap()[:, ts(db, P)].rearrange("b p -> p b").bitcast(F32R),
                )

            encnat_all = {}

            def load_enc(b, tb):
                t_enc = encnat_p.tile([P, E], F32R, tag="encnat", name=f"enc_{b}_{tb}")
                nc.sync.dma_start(t_enc[:], enc_d.ap()[b, ts(tb, P), :].bitcast(F32R))
                encnat_all[(b, tb)] = t_enc

            # issue the first W row-block loads ahead of everything (the first
            # energy matmuls are gated on W_encT availability), then prefetch
            # the first pairs' enc tiles to overlap the rest of W setup
            wn_all = {}

            def load_wn(ab, half):
                # half 1 = W_enc columns (feeds energy GEMM), half 0 = W_dec
                lo, width = (D, E) if half else (0, D)
                wn = wnat_p.tile([P, width], F32R, tag="wnat", name=f"wn{ab}_{half}")
                nc.sync.dma_start(
                    wn[:], W_d.ap()[ts(ab, P), lo:lo + width].bitcast(F32R)
                )
                wn_all[(ab, half)] = wn

            for ab in range(min(AB_GRP + 1, n_ab)):
                load_wn(ab, 1)

            _pairs0 = [(b, tcn) for b in range(B_loc) for tcn in range(n_tc)][:2]
            for b0, tc0 in _pairs0:
                for tbl in range(tb_per_tc):
                    tb0 = tc0 * tb_per_tc + tbl
                    if (b0, tb0) not in encnat_all:
                        load_enc(b0, tb0)

            # ---- W transpose + hb[a, b] = W_dec @ hidden.T + b_attn ----
            # wenc tiles split per (eb, abg) so early a-blocks unblock ASAP
            wenc_t = {}
            for eb in range(n_eb):
                for abg in range(n_abg):
                    wenc_t[(eb, abg)] = wenc_p.tile(
                        [P, AB_GRP * P], F32R, tag=f"wenc{eb}_{abg}", name=f"wenc{eb}_{abg}"
                    )
            hb_all = const.tile([P, n_ab * B_loc], F32, name="hb_all")

            def emit_w_abg(abg):
                for abl in range(AB_GRP):
                    ab = abg * AB_GRP + abl
                    for half in (1, 0):
                        if (ab, half) not in wn_all:
                            load_wn(ab, half)
                ps_hb = [
                    psH.tile([P, B_loc], F32, tag="hb", name=f"pshb{abg}_{abl}")
                    for abl in range(AB_GRP)
                ]
                # W_enc f-blocks first (unblock energy matmuls), then W_dec
                for fb in list(range(n_db, n_db + n_eb)) + list(range(n_db)):
                    half = 1 if fb >= n_db else 0
                    col = fb - n_db if fb >= n_db else fb
                    pw = psT.tile([P, AB_GRP * P], F32R, tag="t", name=f"pw{abg}_{fb}")
                    for abl in range(AB_GRP):
                        ab = abg * AB_GRP + abl
                        nc.tensor.transpose(
                            pw[:, ts(abl, P)],
                            wn_all[(ab, half)][:, ts(col, P)],
                            ident[:],
                        )
                    if fb < n_db:
                        db = fb
                        wd = wdec_p.tile([P, AB_GRP * P], F32R, tag="wdec", name=f"wd{abg}_{db}")
                        nc.vector.tensor_copy(wd[:], pw[:])
                        for abl in range(AB_GRP):
                            nc.tensor.matmul(
                                ps_hb[abl][:],
                                wd[:, ts(abl, P)],
                                hidT[:, db],
                                start=(db == 0),
                                stop=False,
                                skip_group_check=True,
                            )
                    else:
                        eb = fb - n_db
                        if eb % 2 == 0:
                            nc.vector.tensor_copy(wenc_t[(eb, abg)][:], pw[:])
                        else:
                            nc.scalar.copy(wenc_t[(eb, abg)][:], pw[:])
                # + b_attn (rank-1, plain fp32)
                for abl in range(AB_GRP):
                    ab = abg * AB_GRP + abl
                    nc.tensor.matmul(
                        ps_hb[abl][:],
                        b_row[0:1, ts(ab, P)],
                        ones_row[:],
                        start=False,
                        stop=True,
                        skip_group_check=True,
                    )
                    nc.vector.tensor_copy(
                        hb_all[:, ab * B_loc:(ab + 1) * B_loc], ps_hb[abl][:]
                    )

            w_emitted = set()

            def ensure_w(abg):
                if abg not in w_emitted:
                    w_emitted.add(abg)
                    emit_w_abg(abg)

            for _abg in range(min(2, n_abg)):
                ensure_w(_abg)

            # ---- main loop (software-pipelined emission) ----
            # per pair i: energy/tanh/scores for pair i, interleaved with the
            # transposes for pair i+1 and the deferred context matmuls of the
            # previous batch, so the PE queue never runs dry at batch edges.
            pairs = [(b, tcn) for b in range(B_loc) for tcn in range(n_tc)]
            encT_cur: list = []
            encT_next: list = []
            scores_rows = {}
            pending_ctx = None  # (b, wc, rc, ctx_row)

            def emit_transposes(b, tcn, eb):
                pt = psT.tile([P, TCW], F32R, tag="t", name=f"pt{b}_{tcn}_{eb}")
                for tbl in range(tb_per_tc):
                    tb = tcn * tb_per_tc + tbl
                    nc.tensor.transpose(
                        pt[:, ts(tbl, P)], encnat_all[(b, tb)][:, ts(eb, P)], ident[:]
                    )
                et = encT_p.tile([P, TCW], F32R, tag="encT", name=f"encT{b}_{tcn}_{eb}")
                if eb % 2 == 0:
                    nc.vector.tensor_copy(et[:], pt[:])
                else:
                    nc.scalar.copy(et[:], pt[:])
                return et

            def emit_ctx_chunk(bb, wc, rc, ctx_row, ec):
                ps_c = psC.tile([1, ECW], F32, tag="c", name=f"psc{bb}_{ec}")
                for tb in range(n_tb):
                    nc.tensor.matmul(
                        ps_c[:],
                        wc[:, tb:tb + 1],
                        encnat_all[(bb, tb)][:, ts(ec, ECW)],
                        start=(tb == 0),
                        stop=(tb == n_tb - 1),
                        skip_group_check=True,
                    )
                nc.vector.tensor_scalar_mul(
                    ctx_row[0:1, ts(ec, ECW)], ps_c[:], rc[0:1, 0:1]
                )

            def finish_ctx(pend):
                bb, wc, rc, ctx_row, done = pend
                for ec in range(done, n_ec):
                    emit_ctx_chunk(bb, wc, rc, ctx_row, ec)
                nc.sync.dma_start(ctx_d.ap()[bb:bb + 1, :], ctx_row[:])
                for tb in range(n_tb):
                    del encnat_all[(bb, tb)]

            for i, (b, tcn) in enumerate(pairs):
                # issue loads two pairs ahead
                nxt2 = i + 2
                if nxt2 < len(pairs):
                    b2, tcn2 = pairs[nxt2]
                    for tbl in range(tb_per_tc):
                        tb2 = tcn2 * tb_per_tc + tbl
                        if (b2, tb2) not in encnat_all:
                            load_enc(b2, tb2)
                if i == 0:
                    encT_cur = [emit_transposes(b, tcn, eb) for eb in range(n_eb)]
                if tcn == 0:
                    scores_rows[b] = soft_p.tile(
                        [1, S_], F32, tag="scores", name=f"scores{b}"
                    )
                scores_row = scores_rows[b]

                ps_s = psS.tile([1, TCW], F32, tag="s", name=f"pss{b}_{tcn}")
                encT_next = []
                for ab in range(n_ab):
                    if i == 0:
                        ensure_w(min(ab // AB_GRP + 1, n_abg - 1))
                        ensure_w(ab // AB_GRP)
                    ps_e = psE.tile([P, TCW], F32, tag="e", name=f"pse{b}_{tcn}_{ab}")
                    for eb in range(n_eb):
                        nc.tensor.matmul(
                            ps_e[:],
                            wenc_t[(eb, ab // AB_GRP)][:, ts(ab % AB_GRP, P)],
                            encT_cur[eb][:],
                            start=(eb == 0),
                            stop=(eb == n_eb - 1),
                        )
                    th = tanh_p.tile([P, TCW], F32R, tag="tanh", name=f"th{b}_{tcn}_{ab}")
                    nc.scalar.activation(
                        th[:], ps_e[:], AF.Tanh,
                        bias=hb_all[:, ab * B_loc + b: ab * B_loc + b + 1],
                    )
                    nc.tensor.matmul(
                        ps_s[:],
                        vcol[:, ab:ab + 1],
                        th[:],
                        start=(ab == 0),
                        stop=(ab == n_ab - 1),
                        skip_group_check=True,
                    )
                    # interleave next pair's transposes
                    if ab < n_eb and i + 1 < len(pairs):
                        bn, tcnn = pairs[i + 1]
                        encT_next.append(emit_transposes(bn, tcnn, ab))
                    # interleave previous batch's context matmuls
                    if pending_ctx is not None and ab >= 2 and pending_ctx[4] < n_ec:
                        bb, wc, rc, ctx_row, done = pending_ctx
                        emit_ctx_chunk(bb, wc, rc, ctx_row, done)
                        pending_ctx = (bb, wc, rc, ctx_row, done + 1)
                nc.vector.tensor_copy(scores_row[0:1, ts(tcn, TCW)], ps_s[:])
                encT_cur = encT_next

                if pending_ctx is not None and pending_ctx[4] >= n_ec:
                    bb, wc, rc, ctx_row, done = pending_ctx
                    nc.sync.dma_start(ctx_d.ap()[bb:bb + 1, :], ctx_row[:])
                    for tb in range(n_tb):
                        del encnat_all[(bb, tb)]
                    pending_ctx = None

                if tcn == n_tc - 1:
                    # ---- per-batch softmax; ctx matmuls deferred ----
                    row = scores_row[0:1, :]
                    nm = soft_p.tile([1, 1], F32, tag="nm", name=f"nm{b}")
                    nc.vector.reduce_max(nm[:], row, axis=AX.X, negate=True)
                    ew = soft_p.tile([1, S_], F32, tag="ew", name=f"ew{b}")
                    nc.scalar.activation(ew[:], row, AF.Exp, bias=nm[0:1, 0:1])
                    sm = soft_p.tile([1, 1], F32, tag="sm", name=f"sm{b}")
                    nc.vector.reduce_sum(sm[:], ew[:], axis=AX.X)
                    rc = soft_p.tile([1, 1], F32, tag="rc", name=f"rc{b}")
                    nc.vector.reciprocal(rc[:], sm[:])
                    nc.sync.dma_start(ew_d.ap()[b:b + 1, :], ew[:])
                    wc = soft_p.tile([P, n_tb], F32R, tag="wc", name=f"wc{b}")
                    nc.sync.dma_start(
                        wc[:],
                        ew_d.ap()[b].rearrange("(j p) -> p j", p=P).bitcast(F32R),
                    )
                    ctx_row = soft_p.tile([1, E], F32, tag="ctxrow", name=f"ctxrow{b}")
                    if pending_ctx is not None:
                        finish_ctx(pending_ctx)
                    pending_ctx = (b, wc, rc, ctx_row, 0)

            if pending_ctx is not None:
                finish_ctx(pending_ctx)

    nc.compile()
    return nc


def _get_nc():
    key = (B_LOC, S, ENC, ATTN, DEC)
    if key not in _CACHE:
        _CACHE[key] = build_nc(*key)
    return _CACHE[key]


def kernel(hidden, encoder_outputs, W_attn, b_attn, v):
    from concourse.bass_utils import run_bass_kernel_spmd

    hidden = np.ascontiguousarray(np.asarray(hidden, dtype=np.float32))
    enc = np.ascontiguousarray(np.asarray(encoder_outputs, dtype=np.float32))
    W = np.ascontiguousarray(np.asarray(W_attn, dtype=np.float32))
    b = np.ascontiguousarray(np.asarray(b_attn, dtype=np.float32))
    vv = np.ascontiguousarray(np.asarray(v, dtype=np.float32))

    nc = _get_nc()
    in_maps = [
        {
            "hidden": hidden[c * B_LOC:(c + 1) * B_LOC],
            "enc": enc[c * B_LOC:(c + 1) * B_LOC],
            "W": W,
            "b_attn": b,
            "v": vv,
        }
        for c in range(N_CORES)
    ]
    res = run_bass_kernel_spmd(nc, in_maps, core_ids=list(range(N_CORES)))
    out = np.concatenate([res.results[c]["ctx"] for c in range(N_CORES)], axis=0)
    return out.reshape(B, 1, ENC).astype(np.float32)
